# revision 35
# baseline (speedup 1.0000x reference)
"""CrossAttention Trainium2 kernel (8 NeuronCores, SPMD, no collectives).

Shapes: B=4, LQ=1024, LK=2048, QD=768, KD=VD=512, H=1024, NH=16, HD=64.
Sharding: core c = (b = c//2, head-half hh = c%2): each core computes the
full LQ=1024 queries of batch b for its 8 heads (H slice hh*512..+512).
No duplicated work: q/k/v projections are computed only for the core's own
H slice.  The output projection is a partial sum over the H contraction --
each core emits out_partial = attT.T @ Wo[hslice, :]; the host adds the two
partials per batch (+ bo).

Device-side dataflow per core (matmul operands bf16, fp32 PSUM accum):
  qp = (Wq_s.T @ query.T)       [512, 1024]   "q^T" for the head slice
  kp = (Wk_s.T @ key.T)         [512, 2048]
  vp = value @ Wv_s             [2048, 512] (+ ones column per head)
  per head h, key-chunk kc (128 keys):
     S^T = kp_h_chunk.T @ qp_h              (K=64 contraction, [128, 1024])
     es  = exp(S^T * 0.125)                 (ScalarE, PSUM -> SBUF bf16)
     att^T[0:65] += [v_chunk_h | ones].T @ es    (row 64 = softmax denom)
  att_h = att^T[0:64] * recip_approx(denom)     (DVE + DMA broadcast bounce)
  out_partial = att^T_merged.T @ Wo_s       (K=128 chunks over H slice)

Softmax skips max-subtraction: |scores/8| < ~2.5 by construction of the
problem's input scale, so exp is perfectly stable in fp32.
"""
import sys

if "/opt/trn_rl_repo" not in sys.path:
    sys.path.insert(0, "/opt/trn_rl_repo")

import numpy as np
import ml_dtypes

B, LQ, LK = 4, 1024, 2048
QD, KD, VD = 768, 512, 512
H, NH = 1024, 16
HD = H // NH          # 64
HH = H // 2           # 512 H-slice per core
NHC = NH // 2         # 8 heads per core
NCORES = 8

_BF = ml_dtypes.bfloat16
_NC_CACHE = {}


def build_nc():
    import concourse.bacc as bacc
    import concourse.tile as tile
    from concourse import mybir

    f32 = mybir.dt.float32
    bf16 = mybir.dt.bfloat16
    AF = mybir.ActivationFunctionType

    nc = bacc.Bacc("TRN2", target_bir_lowering=False, debug=False)

    # ---- DRAM parameters (per-core views prepared on host) ----
    qT_d = nc.dram_tensor("qT", [QD, LQ], bf16, kind="ExternalInput")
    kT_d = nc.dram_tensor("kT", [KD, LK], bf16, kind="ExternalInput")
    vT_d = nc.dram_tensor("vT", [VD, LK], bf16, kind="ExternalInput")
    wq_d = nc.dram_tensor("wq", [QD, HH], bf16, kind="ExternalInput")
    wk_d = nc.dram_tensor("wk", [KD, HH], bf16, kind="ExternalInput")
    wv_d = nc.dram_tensor("wv", [VD, HH], bf16, kind="ExternalInput")
    wo_d = nc.dram_tensor("wo", [HH, H], bf16, kind="ExternalInput")
    bq_d = nc.dram_tensor("bq", [128, 4], f32, kind="ExternalInput")  # [p, mtile]
    bk_d = nc.dram_tensor("bk", [128, 4], f32, kind="ExternalInput")
    bv_d = nc.dram_tensor("bv", [1, HH], f32, kind="ExternalInput")
    out_d = nc.dram_tensor("out", [LQ, H], f32, kind="ExternalOutput")

    srow_d = nc.dram_tensor("srow", [NHC, LQ], f32)  # per-head recip bounce

    QDC, KDC = QD // 128, KD // 128       # 6, 4 contraction chunks
    MT = HH // 128                        # 4 H-slice tiles
    LKC = LK // 128                       # 16 key chunks
    OMT = LQ // 128                       # 8 out q tiles

    with tile.TileContext(nc) as tc:
        with tc.tile_pool(name="persist", bufs=1) as per, \
             tc.tile_pool(name="es", bufs=22) as esp, \
             tc.tile_pool(name="bc", bufs=2) as bcp, \
             tc.tile_pool(name="rec", bufs=3) as rcp, \
             tc.tile_pool(name="osb", bufs=3) as osp, \
             tc.tile_pool(name="pps", bufs=2, space="PSUM") as pps, \
             tc.tile_pool(name="aps", bufs=2, space="PSUM") as apsp:

            # ---- input loads, spread across the non-scalar DMA queues in
            #      priority order q -> k -> v -> wo ----
            qt = per.tile([128, QDC, LQ], bf16)
            wq = per.tile([128, QDC, HH], bf16)
            bq = per.tile([128, MT], f32)
            kt = per.tile([128, KDC, LK], bf16)
            wk = per.tile([128, KDC, HH], bf16)
            bk = per.tile([128, MT], f32)
            vt = per.tile([128, KDC, LK], bf16)
            wv = per.tile([128, KDC, HH], bf16)
            wo = per.tile([128, MT, H], bf16)
            bv_bc = per.tile([128, HH], f32)

            loads = [(bq[:], bq_d[:]), (bk[:], bk_d[:])]
            for i in range(QDC):
                loads.append((qt[:, i, :], qT_d[i * 128:(i + 1) * 128, :]))
                loads.append((wq[:, i, :], wq_d[i * 128:(i + 1) * 128, :]))
            for i in range(KDC):
                loads.append((kt[:, i, :], kT_d[i * 128:(i + 1) * 128, :]))
                loads.append((wk[:, i, :], wk_d[i * 128:(i + 1) * 128, :]))
            for i in range(KDC):
                loads.append((vt[:, i, :], vT_d[i * 128:(i + 1) * 128, :]))
                loads.append((wv[:, i, :], wv_d[i * 128:(i + 1) * 128, :]))
            for i in range(MT):
                loads.append((wo[:, i, :], wo_d[i * 128:(i + 1) * 128, :]))
            # NOTE: never issue input loads from the scalar queue -- DMA
            # descriptor writes carry ring flow-control waits that would
            # serialize in front of the exp ACTIVATEs in the scalar stream.
            queues = [nc.sync, nc.gpsimd]
            for j, (dst, src) in enumerate(loads):
                queues[j % 2].dma_start(dst, src)
            nc.gpsimd.dma_start(out=bv_bc[:],
                                in_=bv_d[0:1, :].to_broadcast([128, HH]))

            # ---- projection outputs ----
            qp = per.tile([128, MT, LQ], bf16)        # q^T: [512, 1024]
            kp = per.tile([128, MT, LK], bf16)        # k^T: [512, 2048]
            vp_t = [per.tile([128, NHC, HD + 1], bf16, name=f"vp{l}")
                    for l in range(LKC)]
            attT = per.tile([128, MT, LQ], bf16)      # att^T: [512, 1024]
            for l in range(LKC):
                nc.gpsimd.memset(vp_t[l][:, :, HD:HD + 1], 1.0)

            # ---- q projection ----
            for m in range(MT):
                ps = pps.tile([128, 2, 512], f32)
                for kc in range(QDC):
                    for qh in range(2):
                        nc.tensor.matmul(ps[:, qh, :],
                                         wq[:, kc, m * 128:(m + 1) * 128],
                                         qt[:, kc, qh * 512:(qh + 1) * 512],
                                         start=(kc == 0), stop=(kc == QDC - 1))
                nc.vector.tensor_scalar_add(
                    qp[:, m, :], ps[:].rearrange("p a b -> p (a b)"),
                    bq[:, m:m + 1])

            # ---- interleaved k-proj / v-proj / attention ----
            # unit i = (hl, kc): hl local head, kc key chunk; covers both
            # query halves ([128 keys, 1024 q] per unit)
            S_seq = [(hl, kc) for hl in range(NHC) for kc in range(LKC)]
            NS = len(S_seq)               # 128
            LAG = 2
            ES_CAP = 20
            es_tiles = {}
            att_tiles = {}
            state = {"s": 0, "a": 0, "lkm_done": -1, "km_done": -1}

            def emit_S():
                i = state["s"]
                hl, kc = S_seq[i]
                po = 64 * (hl % 2)
                hc = hl // 2
                sps = pps.tile([128, 2, 512], f32, name="sps", tag="ps")
                for qh in range(2):
                    nc.tensor.matmul(sps[:, qh, :],
                                     kp[po:po + 64, hc, kc * 128:(kc + 1) * 128],
                                     qp[po:po + 64, hc, qh * 512:(qh + 1) * 512],
                                     start=True, stop=True)
                es = esp.tile([128, 2, 512], bf16, name="es", tag="es")
                nc.scalar.activation(es[:], sps[:], AF.Exp, scale=0.125)
                es_tiles[i] = es
                state["s"] += 1

            def can_S():
                if state["s"] >= NS:
                    return False
                if state["s"] - state["a"] >= ES_CAP:
                    return False
                hl, kc = S_seq[state["s"]]
                return hl // 2 <= state["km_done"]

            def can_att():
                if state["a"] >= NS or state["a"] > state["s"] - LAG:
                    return False
                hl, kc = S_seq[state["a"]]
                return kc <= state["lkm_done"]

            def emit_att():
                i = state["a"]
                hl, kc = S_seq[i]
                if kc == 0:
                    att_tiles[hl] = apsp.tile([128, 2, 512], f32,
                                              name="attps", tag="attps")
                aps = att_tiles[hl]
                es = es_tiles.pop(i)
                for qh in range(2):
                    nc.tensor.matmul(aps[0:HD + 1, qh, :],
                                     vp_t[kc][:, hl, :],
                                     es[:, qh, :],
                                     start=(kc == 0), stop=(kc == LKC - 1))
                if kc == LKC - 1:
                    # head complete: normalize both query halves.
                    # (denominator row sits at PSUM partition 64; custom-DVE
                    # recip only works at partition base 0, so copy it down.)
                    po = 64 * (hl % 2)
                    hc = hl // 2
                    den = rcp.tile([1, LQ], f32, name="den", tag="den")
                    nc.vector.tensor_copy(
                        den[0:1, :],
                        aps[64:65, :, :].rearrange("p a b -> p (a b)"))
                    rec = rcp.tile([1, LQ], f32, name="rec", tag="rec")
                    nc.vector.reciprocal_approx_fast(
                        out=rec[0:1, :], in_=den[0:1, :])
                    nc.sync.dma_start(out=srow_d[hl:hl + 1, :],
                                      in_=rec[0:1, :])
                    bcst = bcp.tile([64, LQ], f32, name="bcst", tag="bcst")
                    nc.gpsimd.dma_start(
                        out=bcst[:],
                        in_=srow_d[hl:hl + 1, :].to_broadcast([64, LQ]))
                    nc.vector.tensor_mul(
                        attT[po:po + 64, hc, :],
                        aps[0:HD, :, :].rearrange("p a b -> p (a b)"),
                        bcst[:])
                    del att_tiles[hl]
                state["a"] += 1

            # phase K: k-proj with S-unit run-ahead
            for m in range(MT):
                for ng in range(2):
                    ps = pps.tile([128, 2, 512], f32)
                    for kc in range(KDC):
                        for j in range(2):
                            n = 2 * ng + j
                            nc.tensor.matmul(ps[:, j, :],
                                             wk[:, kc, m * 128:(m + 1) * 128],
                                             kt[:, kc, n * 512:(n + 1) * 512],
                                             start=(kc == 0),
                                             stop=(kc == KDC - 1))
                    nc.vector.tensor_scalar_add(
                        kp[:, m, ng * 1024:(ng + 1) * 1024],
                        ps[:].rearrange("p a b -> p (a b)"),
                        bk[:, m:m + 1])
                state["km_done"] = m
                for _ in range(3):
                    if can_S():
                        emit_S()

            # phase V: v-proj (pairs of key chunks) with S + att interleave
            for lp in range(LKC // 2):
                ps = pps.tile([128, 2, 512], f32)
                for kc in range(KDC):
                    for j in range(2):
                        lkm = 2 * lp + j
                        nc.tensor.matmul(ps[:, j, :],
                                         vt[:, kc, lkm * 128:(lkm + 1) * 128],
                                         wv[:, kc, :],
                                         start=(kc == 0), stop=(kc == KDC - 1))
                for j in range(2):
                    lkm = 2 * lp + j
                    nc.vector.tensor_add(
                        vp_t[lkm][:, :, 0:HD],
                        ps[:, j, :].rearrange("p (h d) -> p h d", h=NHC),
                        bv_bc[:].rearrange("p (h d) -> p h d", h=NHC))
                state["lkm_done"] = 2 * lp + 1
                for _ in range(16):
                    progressed = False
                    if can_S():
                        emit_S()
                        progressed = True
                    if can_att():
                        emit_att()
                        progressed = True
                    if not progressed:
                        break

            # drain
            while state["s"] < NS or state["a"] < NS:
                progressed = False
                if can_S():
                    emit_S()
                    progressed = True
                if can_att():
                    emit_att()
                    progressed = True
                if not progressed:
                    if state["a"] < NS and state["a"] < state["s"]:
                        emit_att()
                    elif state["s"] < NS:
                        emit_S()

            # ---- output projection: out[m] = attT[:,m].T @ wo  (no bias;
            #      host adds bo after summing the two partials) ----
            for m in range(OMT):
                osb = osp.tile([128, H], f32)
                ps = pps.tile([128, 2, 512], f32)
                for kc in range(MT):
                    for n2 in range(2):
                        nc.tensor.matmul(ps[:, n2, :],
                                         attT[:, kc, m * 128:(m + 1) * 128],
                                         wo[:, kc, n2 * 512:(n2 + 1) * 512],
                                         start=(kc == 0), stop=(kc == MT - 1))
                if m % 2 == 0:
                    nc.scalar.copy(osb[:], ps[:].rearrange("p a b -> p (a b)"))
                else:
                    nc.vector.tensor_copy(osb[:],
                                          ps[:].rearrange("p a b -> p (a b)"))
                q = nc.sync if m % 2 == 0 else nc.scalar
                q.dma_start(out_d[m * 128:(m + 1) * 128, :], osb[:])

    nc.compile()
    return nc


def _get_nc():
    if "nc" not in _NC_CACHE:
        _NC_CACHE["nc"] = build_nc()
    return _NC_CACHE["nc"]


MT_ = HH // 128


def make_in_maps(query, key, value, Wq, bq, Wk, bk, Wv, bv, Wo, bo):
    query = np.asarray(query, np.float32)
    key = np.asarray(key, np.float32)
    value = np.asarray(value, np.float32)
    Wq = np.asarray(Wq, np.float32)
    Wk = np.asarray(Wk, np.float32)
    Wv = np.asarray(Wv, np.float32)
    Wo = np.asarray(Wo, np.float32)
    bq = np.asarray(bq, np.float32)
    bk = np.asarray(bk, np.float32)
    bv = np.asarray(bv, np.float32)

    qT = [np.ascontiguousarray(query[b].T.astype(_BF)) for b in range(B)]
    kT = [np.ascontiguousarray(key[b].T.astype(_BF)) for b in range(B)]
    vT = [np.ascontiguousarray(value[b].T.astype(_BF)) for b in range(B)]

    half = {}
    for hh in range(2):
        s = slice(hh * HH, (hh + 1) * HH)
        half[hh] = {
            "wq": np.ascontiguousarray(Wq[:, s].astype(_BF)),
            "wk": np.ascontiguousarray(Wk[:, s].astype(_BF)),
            "wv": np.ascontiguousarray(Wv[:, s].astype(_BF)),
            "wo": np.ascontiguousarray(Wo[s, :].astype(_BF)),
            "bq": np.ascontiguousarray(bq[s].reshape(MT_, 128).T),
            "bk": np.ascontiguousarray(bk[s].reshape(MT_, 128).T),
            "bv": bv[s].reshape(1, HH).copy(),
        }
    in_maps = []
    for c in range(NCORES):
        b, hh = divmod(c, 2)
        m = dict(half[hh])
        m["qT"] = qT[b]
        m["kT"] = kT[b]
        m["vT"] = vT[b]
        in_maps.append(m)
    return in_maps


def run(inputs, trace=False):
    from concourse.bass_utils import run_bass_kernel_spmd

    nc = _get_nc()
    in_maps = make_in_maps(**inputs)
    res = run_bass_kernel_spmd(nc, in_maps, list(range(NCORES)), trace=trace)
    bo = np.asarray(inputs["bo"], np.float32).reshape(1, H)
    out = np.empty((B, LQ, H), np.float32)
    for b in range(B):
        out[b] = res.results[2 * b]["out"] + res.results[2 * b + 1]["out"] + bo
    return out, res


def kernel(**inputs):
    out, _ = run(inputs, trace=False)
    return out


# revision 36
# speedup vs baseline: 1.0423x; 1.0423x over previous
"""CrossAttention Trainium2 kernel (8 NeuronCores, SPMD, no collectives).

Shapes: B=4, LQ=1024, LK=2048, QD=768, KD=VD=512, H=1024, NH=16, HD=64.
Sharding: core c = (b = c//2, head-half hh = c%2): each core computes the
full LQ=1024 queries of batch b for its 8 heads (H slice hh*512..+512).
No duplicated work: q/k/v projections are computed only for the core's own
H slice.  The output projection is a partial sum over the H contraction --
each core emits out_partial = attT.T @ Wo[hslice, :]; the host adds the two
partials per batch (+ bo).

Device-side dataflow per core (matmul operands bf16, fp32 PSUM accum):
  qp = (Wq_s.T @ query.T)       [512, 1024]   "q^T" for the head slice
  kp = (Wk_s.T @ key.T)         [512, 2048]
  vp = value @ Wv_s             [2048, 512] (+ ones column per head)
  per head h, key-chunk kc (128 keys):
     S^T = kp_h_chunk.T @ qp_h              (K=64 contraction, [128, 1024])
     es  = exp(S^T * 0.125)                 (ScalarE, PSUM -> SBUF bf16)
     att^T[0:65] += [v_chunk_h | ones].T @ es    (row 64 = softmax denom)
  att_h = att^T[0:64] * recip_approx(denom)     (DVE + DMA broadcast bounce)
  out_partial = att^T_merged.T @ Wo_s       (K=128 chunks over H slice)

Softmax skips max-subtraction: |scores/8| < ~2.5 by construction of the
problem's input scale, so exp is perfectly stable in fp32.
"""
import sys

if "/opt/trn_rl_repo" not in sys.path:
    sys.path.insert(0, "/opt/trn_rl_repo")

import numpy as np
import ml_dtypes

B, LQ, LK = 4, 1024, 2048
QD, KD, VD = 768, 512, 512
H, NH = 1024, 16
HD = H // NH          # 64
HH = H // 2           # 512 H-slice per core
NHC = NH // 2         # 8 heads per core
NCORES = 8

_BF = ml_dtypes.bfloat16
_NC_CACHE = {}


def build_nc():
    import concourse.bacc as bacc
    import concourse.tile as tile
    from concourse import mybir

    f32 = mybir.dt.float32
    bf16 = mybir.dt.bfloat16
    AF = mybir.ActivationFunctionType

    nc = bacc.Bacc("TRN2", target_bir_lowering=False, debug=False)

    # ---- DRAM parameters (per-core views prepared on host) ----
    qT_d = nc.dram_tensor("qT", [QD, LQ], bf16, kind="ExternalInput")
    kT_d = nc.dram_tensor("kT", [KD, LK], bf16, kind="ExternalInput")
    vT_d = nc.dram_tensor("vT", [VD, LK], bf16, kind="ExternalInput")
    wq_d = nc.dram_tensor("wq", [QD, HH], bf16, kind="ExternalInput")
    wk_d = nc.dram_tensor("wk", [KD, HH], bf16, kind="ExternalInput")
    wv_d = nc.dram_tensor("wv", [VD, HH], bf16, kind="ExternalInput")
    wo_d = nc.dram_tensor("wo", [HH, H], bf16, kind="ExternalInput")
    bq_d = nc.dram_tensor("bq", [128, 4], f32, kind="ExternalInput")  # [p, mtile]
    bk_d = nc.dram_tensor("bk", [128, 4], f32, kind="ExternalInput")
    bv_d = nc.dram_tensor("bv", [1, HH], f32, kind="ExternalInput")
    out_d = nc.dram_tensor("out", [LQ, H], f32, kind="ExternalOutput")

    srow_d = nc.dram_tensor("srow", [NHC, LQ], f32)  # per-head recip bounce

    QDC, KDC = QD // 128, KD // 128       # 6, 4 contraction chunks
    MT = HH // 128                        # 4 H-slice tiles
    LKC = LK // 128                       # 16 key chunks
    OMT = LQ // 128                       # 8 out q tiles

    with tile.TileContext(nc) as tc:
        with tc.tile_pool(name="persist", bufs=1) as per, \
             tc.tile_pool(name="es", bufs=16) as esp, \
             tc.tile_pool(name="bc", bufs=2) as bcp, \
             tc.tile_pool(name="rec", bufs=3) as rcp, \
             tc.tile_pool(name="osb", bufs=3) as osp, \
             tc.tile_pool(name="pps", bufs=2, space="PSUM") as pps, \
             tc.tile_pool(name="aps", bufs=2, space="PSUM") as apsp:

            # ---- input loads, spread across the non-scalar DMA queues in
            #      priority order q -> k -> v -> wo ----
            qt = per.tile([128, QDC, LQ], bf16)
            wq = per.tile([128, QDC, HH], bf16)
            bq = per.tile([128, MT], f32)
            kt = per.tile([128, KDC, LK], bf16)
            wk = per.tile([128, KDC, HH], bf16)
            bk = per.tile([128, MT], f32)
            vt = per.tile([128, KDC, LK], bf16)
            wv = per.tile([128, KDC, HH], bf16)
            wo = per.tile([128, MT, H], bf16)
            bv_bc = per.tile([128, HH], f32)

            loads = [(bq[:], bq_d[:]), (bk[:], bk_d[:])]
            for i in range(QDC):
                loads.append((qt[:, i, :], qT_d[i * 128:(i + 1) * 128, :]))
                loads.append((wq[:, i, :], wq_d[i * 128:(i + 1) * 128, :]))
            for i in range(KDC):
                loads.append((kt[:, i, :], kT_d[i * 128:(i + 1) * 128, :]))
                loads.append((wk[:, i, :], wk_d[i * 128:(i + 1) * 128, :]))
            for i in range(KDC):
                loads.append((vt[:, i, :], vT_d[i * 128:(i + 1) * 128, :]))
                loads.append((wv[:, i, :], wv_d[i * 128:(i + 1) * 128, :]))
            for i in range(MT):
                loads.append((wo[:, i, :], wo_d[i * 128:(i + 1) * 128, :]))
            # NOTE: never issue input loads from the scalar queue -- DMA
            # descriptor writes carry ring flow-control waits that would
            # serialize in front of the exp ACTIVATEs in the scalar stream.
            queues = [nc.sync, nc.gpsimd]
            for j, (dst, src) in enumerate(loads):
                queues[j % 2].dma_start(dst, src)
            nc.gpsimd.dma_start(out=bv_bc[:],
                                in_=bv_d[0:1, :].to_broadcast([128, HH]))

            # ---- projection outputs ----
            qp = per.tile([128, MT, LQ], bf16)        # q^T: [512, 1024]
            kp = per.tile([128, MT, LK], bf16)        # k^T: [512, 2048]
            vp_t = [per.tile([128, NHC, HD + 1], bf16, name=f"vp{l}")
                    for l in range(LKC)]
            attT = per.tile([128, MT, LQ], bf16)      # att^T: [512, 1024]
            for l in range(LKC):
                nc.gpsimd.memset(vp_t[l][:, :, HD:HD + 1], 1.0)

            # ---- q projection ----
            for m in range(MT):
                ps = pps.tile([128, 2, 512], f32)
                for kc in range(QDC):
                    for qh in range(2):
                        nc.tensor.matmul(ps[:, qh, :],
                                         wq[:, kc, m * 128:(m + 1) * 128],
                                         qt[:, kc, qh * 512:(qh + 1) * 512],
                                         start=(kc == 0), stop=(kc == QDC - 1))
                nc.vector.tensor_scalar_add(
                    qp[:, m, :], ps[:].rearrange("p a b -> p (a b)"),
                    bq[:, m:m + 1])

            # ---- interleaved k-proj / v-proj / attention ----
            # unit i = (hl, kc): hl local head, kc key chunk; covers both
            # query halves ([128 keys, 1024 q] per unit)
            S_seq = [(hl, kc) for hl in range(NHC) for kc in range(LKC)]
            NS = len(S_seq)               # 128
            LAG = 2
            ES_CAP = 14
            es_tiles = {}
            att_tiles = {}
            state = {"s": 0, "a": 0, "lkm_done": -1, "km_done": -1}

            def emit_S():
                i = state["s"]
                hl, kc = S_seq[i]
                po = 64 * (hl % 2)
                hc = hl // 2
                sps = pps.tile([128, 2, 512], f32, name="sps", tag="ps")
                for qh in range(2):
                    nc.tensor.matmul(sps[:, qh, :],
                                     kp[po:po + 64, hc, kc * 128:(kc + 1) * 128],
                                     qp[po:po + 64, hc, qh * 512:(qh + 1) * 512],
                                     start=True, stop=True)
                es = esp.tile([128, 2, 512], bf16, name="es", tag="es")
                nc.scalar.activation(es[:], sps[:], AF.Exp, scale=0.125)
                es_tiles[i] = es
                state["s"] += 1

            def can_S():
                if state["s"] >= NS:
                    return False
                if state["s"] - state["a"] >= ES_CAP:
                    return False
                hl, kc = S_seq[state["s"]]
                return hl // 2 <= state["km_done"]

            def can_att():
                if state["a"] >= NS or state["a"] > state["s"] - LAG:
                    return False
                hl, kc = S_seq[state["a"]]
                return kc <= state["lkm_done"]

            def emit_att():
                i = state["a"]
                hl, kc = S_seq[i]
                if kc == 0:
                    att_tiles[hl] = apsp.tile([128, 2, 512], f32,
                                              name="attps", tag="attps")
                aps = att_tiles[hl]
                es = es_tiles.pop(i)
                for qh in range(2):
                    nc.tensor.matmul(aps[0:HD + 1, qh, :],
                                     vp_t[kc][:, hl, :],
                                     es[:, qh, :],
                                     start=(kc == 0), stop=(kc == LKC - 1))
                if kc == LKC - 1:
                    # head complete: normalize both query halves.
                    # (denominator row sits at PSUM partition 64; custom-DVE
                    # recip only works at partition base 0, so copy it down.)
                    po = 64 * (hl % 2)
                    hc = hl // 2
                    den = rcp.tile([1, LQ], f32, name="den", tag="den")
                    nc.vector.tensor_copy(
                        den[0:1, :],
                        aps[64:65, :, :].rearrange("p a b -> p (a b)"))
                    rec = rcp.tile([1, LQ], f32, name="rec", tag="rec")
                    nc.vector.reciprocal_approx_fast(
                        out=rec[0:1, :], in_=den[0:1, :])
                    nc.sync.dma_start(out=srow_d[hl:hl + 1, :],
                                      in_=rec[0:1, :])
                    bcst = bcp.tile([64, LQ], f32, name="bcst", tag="bcst")
                    nc.gpsimd.dma_start(
                        out=bcst[:],
                        in_=srow_d[hl:hl + 1, :].to_broadcast([64, LQ]))
                    nc.vector.tensor_mul(
                        attT[po:po + 64, hc, :],
                        aps[0:HD, :, :].rearrange("p a b -> p (a b)"),
                        bcst[:])
                    del att_tiles[hl]
                state["a"] += 1

            # phase K: k-proj with S-unit run-ahead
            for m in range(MT):
                for ng in range(2):
                    ps = pps.tile([128, 2, 512], f32)
                    for kc in range(KDC):
                        for j in range(2):
                            n = 2 * ng + j
                            nc.tensor.matmul(ps[:, j, :],
                                             wk[:, kc, m * 128:(m + 1) * 128],
                                             kt[:, kc, n * 512:(n + 1) * 512],
                                             start=(kc == 0),
                                             stop=(kc == KDC - 1))
                    nc.vector.tensor_scalar_add(
                        kp[:, m, ng * 1024:(ng + 1) * 1024],
                        ps[:].rearrange("p a b -> p (a b)"),
                        bk[:, m:m + 1])
                state["km_done"] = m
                for _ in range(3):
                    if can_S():
                        emit_S()

            # phase V: v-proj (pairs of key chunks) with S + att interleave
            for lp in range(LKC // 2):
                ps = pps.tile([128, 2, 512], f32)
                for kc in range(KDC):
                    for j in range(2):
                        lkm = 2 * lp + j
                        nc.tensor.matmul(ps[:, j, :],
                                         vt[:, kc, lkm * 128:(lkm + 1) * 128],
                                         wv[:, kc, :],
                                         start=(kc == 0), stop=(kc == KDC - 1))
                for j in range(2):
                    lkm = 2 * lp + j
                    nc.vector.tensor_add(
                        vp_t[lkm][:, :, 0:HD],
                        ps[:, j, :].rearrange("p (h d) -> p h d", h=NHC),
                        bv_bc[:].rearrange("p (h d) -> p h d", h=NHC))
                state["lkm_done"] = 2 * lp + 1
                for _ in range(16):
                    progressed = False
                    if can_S():
                        emit_S()
                        progressed = True
                    if can_att():
                        emit_att()
                        progressed = True
                    if not progressed:
                        break

            # drain
            while state["s"] < NS or state["a"] < NS:
                progressed = False
                if can_S():
                    emit_S()
                    progressed = True
                if can_att():
                    emit_att()
                    progressed = True
                if not progressed:
                    if state["a"] < NS and state["a"] < state["s"]:
                        emit_att()
                    elif state["s"] < NS:
                        emit_S()

            # ---- output projection: out[m] = attT[:,m].T @ wo  (no bias;
            #      host adds bo after summing the two partials) ----
            for m in range(OMT):
                osb = osp.tile([128, H], f32)
                ps = pps.tile([128, 2, 512], f32)
                for kc in range(MT):
                    for n2 in range(2):
                        nc.tensor.matmul(ps[:, n2, :],
                                         attT[:, kc, m * 128:(m + 1) * 128],
                                         wo[:, kc, n2 * 512:(n2 + 1) * 512],
                                         start=(kc == 0), stop=(kc == MT - 1))
                if m % 2 == 0:
                    nc.scalar.copy(osb[:], ps[:].rearrange("p a b -> p (a b)"))
                else:
                    nc.vector.tensor_copy(osb[:],
                                          ps[:].rearrange("p a b -> p (a b)"))
                q = nc.sync if m % 2 == 0 else nc.scalar
                q.dma_start(out_d[m * 128:(m + 1) * 128, :], osb[:])

    nc.compile()
    return nc


def _get_nc():
    if "nc" not in _NC_CACHE:
        _NC_CACHE["nc"] = build_nc()
    return _NC_CACHE["nc"]


MT_ = HH // 128


def make_in_maps(query, key, value, Wq, bq, Wk, bk, Wv, bv, Wo, bo):
    query = np.asarray(query, np.float32)
    key = np.asarray(key, np.float32)
    value = np.asarray(value, np.float32)
    Wq = np.asarray(Wq, np.float32)
    Wk = np.asarray(Wk, np.float32)
    Wv = np.asarray(Wv, np.float32)
    Wo = np.asarray(Wo, np.float32)
    bq = np.asarray(bq, np.float32)
    bk = np.asarray(bk, np.float32)
    bv = np.asarray(bv, np.float32)

    qT = [np.ascontiguousarray(query[b].T.astype(_BF)) for b in range(B)]
    kT = [np.ascontiguousarray(key[b].T.astype(_BF)) for b in range(B)]
    vT = [np.ascontiguousarray(value[b].T.astype(_BF)) for b in range(B)]

    half = {}
    for hh in range(2):
        s = slice(hh * HH, (hh + 1) * HH)
        half[hh] = {
            "wq": np.ascontiguousarray(Wq[:, s].astype(_BF)),
            "wk": np.ascontiguousarray(Wk[:, s].astype(_BF)),
            "wv": np.ascontiguousarray(Wv[:, s].astype(_BF)),
            "wo": np.ascontiguousarray(Wo[s, :].astype(_BF)),
            "bq": np.ascontiguousarray(bq[s].reshape(MT_, 128).T),
            "bk": np.ascontiguousarray(bk[s].reshape(MT_, 128).T),
            "bv": bv[s].reshape(1, HH).copy(),
        }
    in_maps = []
    for c in range(NCORES):
        b, hh = divmod(c, 2)
        m = dict(half[hh])
        m["qT"] = qT[b]
        m["kT"] = kT[b]
        m["vT"] = vT[b]
        in_maps.append(m)
    return in_maps


def run(inputs, trace=False):
    from concourse.bass_utils import run_bass_kernel_spmd

    nc = _get_nc()
    in_maps = make_in_maps(**inputs)
    res = run_bass_kernel_spmd(nc, in_maps, list(range(NCORES)), trace=trace)
    bo = np.asarray(inputs["bo"], np.float32).reshape(1, H)
    out = np.empty((B, LQ, H), np.float32)
    for b in range(B):
        out[b] = res.results[2 * b]["out"] + res.results[2 * b + 1]["out"] + bo
    return out, res


def kernel(**inputs):
    out, _ = run(inputs, trace=False)
    return out
